# revision 23
# baseline (speedup 1.0000x reference)
"""Trainium2 Bass kernel for nn_MultiHeadAttention_343597384205.

Reference computation (B=2, S=4096, d_model=4096, H=32 heads, D=128):
    q/k/v = per-head shared linear projections of query/key/value
    energy = einsum("bshd,bstd->bsht", q, k) / sqrt(d_model)   (HxH per token)
    attn   = softmax(energy + mask, axis=-1)
    out    = einsum("bsht,bstd->bshd", attn, v) @ Wo.T + bo

Strategy: data parallelism over the 8192 tokens across 8 NeuronCores (1024
tokens/core; the HxH attention is per-token).  Weight folding removes two of
the three projection GEMMs on device:
    C  = Wq^T Wk / sqrt(d_model)      ->  E[h,t'] = xq_h . (C xk_t')
    W2 = blockdiag(Wv^T) @ Wo^T       ->  y = concat_h(sum_t' A[h,t'] xv_t') @ W2
Per-core pipeline (64-token groups):
    kkT  = C-projection of xkT                       (4 N=512 matmuls)
    E^T  = per-token KKT^T xqT (col-tiled 32x32 into one PSUM bank)
    A^T  = exp(E+mask) * recip(blockdiag-ones-matmul sums)   (ACT+DVE+1 MM)
    mixT = per-token XV^T A^T (row-tiled, raw xv stationary)  -> attnT[e,h,t]
Output GEMM: yT[f,t] = sum_h W2(h)^T @ attnT[:,h,:] + bo, streamed over W2
tiles in 2-PSUM-bank chunks interleaved with the attention groups.
All matmuls bf16 with fp32 PSUM accumulation.
"""
import numpy as np
import ml_dtypes
from contextlib import ExitStack

BF = ml_dtypes.bfloat16

N_CORES = 8
D_MODEL = 4096
H = 32
D = 128
FCH_ = D_MODEL // 128


def _build(TPC, repeat=1, parts="all"):
    import concourse.bacc as bacc
    import concourse.mybir as mybir
    import concourse.tile as tile

    F32 = mybir.dt.float32
    BF16 = mybir.dt.bfloat16
    AF = mybir.ActivationFunctionType

    G = TPC // 64
    assert TPC % 512 == 0 and G >= 8
    TH = TPC // 512
    nc = bacc.Bacc("TRN2", target_bir_lowering=False, debug=False)

    xqb = nc.dram_tensor("xqb", [D, TPC * H], BF16, kind="ExternalInput").ap()
    xkb = nc.dram_tensor("xkb", [D, TPC * H], BF16, kind="ExternalInput").ap()
    # xv in partition-blocked natural layout: [p=(tok%4)*32+t', (tok//4)*128+e]
    xvb = nc.dram_tensor("xvb", [D, TPC * H], BF16, kind="ExternalInput").ap()
    ct = nc.dram_tensor("ct", [D, D], BF16, kind="ExternalInput").ap()
    blkd = nc.dram_tensor("blkd", [D, D], BF16, kind="ExternalInput").ap()
    maskt = nc.dram_tensor("maskt", [128, 512], F32, kind="ExternalInput").ap()
    # W2 re-laid-out on host: [fc, e, (h, f')] with f = 128*fc + f'
    w2b = nc.dram_tensor("w2b", [D_MODEL // 128, 128, H * 128], BF16,
                         kind="ExternalInput").ap()
    bo32 = nc.dram_tensor("bo32", [128, FCH_], F32, kind="ExternalInput").ap()
    yT = nc.dram_tensor("yT", [D_MODEL, TPC], BF16, kind="ExternalOutput").ap()

    with tile.TileContext(nc) as tc, ExitStack() as ctx:
        const = ctx.enter_context(tc.tile_pool(name="const", bufs=1))
        xt_pool = ctx.enter_context(tc.tile_pool(name="xt", bufs=2))
        kkt_pool = ctx.enter_context(tc.tile_pool(name="kkt", bufs=2))
        e_ps = ctx.enter_context(tc.tile_pool(name="e_ps", bufs=2, space="PSUM"))
        mx_ps = ctx.enter_context(tc.tile_pool(name="mx_ps", bufs=4, space="PSUM"))
        mm_ps = ctx.enter_context(tc.tile_pool(name="mm_ps", bufs=2, space="PSUM"))
        # PSUM budget: 2 (Eps) + 4 (Sps+mix ring) + 2 (kk-proj + p2 chunks) = 8
        sm_sb = ctx.enter_context(tc.tile_pool(name="sm_sb", bufs=2))
        attnT_pool = ctx.enter_context(tc.tile_pool(name="attnT", bufs=1))
        p2_sb = ctx.enter_context(tc.tile_pool(name="p2_sb", bufs=4))
        out_sb = ctx.enter_context(tc.tile_pool(name="out_sb", bufs=3))

        ct_sb = const.tile([D, D], BF16)
        nc.sync.dma_start(ct_sb, ct)
        bd_sb = const.tile([D, D], BF16)
        nc.sync.dma_start(bd_sb, blkd)
        mask_sb = const.tile([128, 512], F32)
        nc.sync.dma_start(mask_sb, maskt)
        bo_sb = const.tile([128, FCH_], F32)
        nc.sync.dma_start(bo_sb, bo32)

        vec, sca = nc.vector, nc.scalar

        def copy_op(eng, out, in_):
            if eng is vec:
                eng.tensor_copy(out=out, in_=in_)
            else:
                eng.copy(out=out, in_=in_)

        # ---------------- phase 1: per 64-token group ----------------
        # front: DMA + kk projection + E^T matmuls (PE-dense, no stalls)
        # back:  softmax (ACT/DVE + sum matmul) + mix matmuls + drains
        # Emitting front(g+1) before back(g) hides the softmax latency of
        # group g behind group g+1's E matmuls.
        def emit_front(g):
            r0 = 2048 * g

            xqT = xt_pool.tile([128, 2048], BF16, tag="xqT")
            nc.sync.dma_start(xqT, xqb[:, r0:r0 + 2048])
            xkT = xt_pool.tile([128, 2048], BF16, tag="xkT")
            nc.sync.dma_start(xkT, xkb[:, r0:r0 + 2048])
            XV = xt_pool.tile([128, 2048], BF16, tag="XV")
            nc.sync.dma_start(XV, xvb[:, r0:r0 + 2048])

            # kk projection: kkT[:, col] = C @ xkT[:, col]
            # (banks from the mx ring; mm_ps is reserved for p2 chunk pairs)
            KKT = kkt_pool.tile([128, 2048], BF16, tag="KKT")
            for s in range(4):
                pp = mx_ps.tile([128, 512], F32, tag="mx", name=f"kk_{g}_{s}")
                nc.tensor.matmul(pp, ct_sb, xkT[:, 512 * s:512 * (s + 1)],
                                 start=True, stop=True)
                copy_op(vec if s % 2 == 0 else sca,
                        KKT[:, 512 * s:512 * (s + 1)], pp)
            if parts == "kk":
                return None

            # E^T: token b at (partition-block b%4, col-slot b//4)
            Eps = e_ps.tile([128, 512], F32, tag="eps", name=f"eps_{g}")
            for b in range(64):
                j, c = b % 4, b // 4
                nc.tensor.matmul(Eps[32 * j:32 * (j + 1), 32 * c:32 * (c + 1)],
                                 KKT[:, 32 * b:32 * (b + 1)],
                                 xqT[:, 32 * b:32 * (b + 1)],
                                 start=True, stop=True,
                                 tile_position=(0, 32 * j))
            return (g, XV, Eps)

        def emit_back(fctx, attnT_halves):
            if fctx is None:
                return
            g, XV, Eps = fctx
            # softmax over t' (partition blocks) without max-subtraction:
            # logits are O(1) for this problem.
            Ez = sm_sb.tile([128, 512], F32, tag="Ez")
            vec.tensor_add(out=Ez, in0=Eps, in1=mask_sb)
            if parts == "E":
                return
            ExpT = sm_sb.tile([128, 512], BF16, tag="ExpT")
            sca.activation(ExpT, Ez, AF.Exp)
            Sps = mx_ps.tile([128, 512], F32, tag="mx", name=f"sps_{g}")
            nc.tensor.matmul(Sps, bd_sb, ExpT, start=True, stop=True)
            Sinv = sm_sb.tile([128, 512], BF16, tag="Sinv")
            with nc.allow_low_precision(reason="softmax sums are O(32); bf16 "
                                        "recip adds ~0.2% rel, validated"):
                vec.reciprocal(Sinv, Sps)
            AT = sm_sb.tile([128, 512], BF16, tag="AT")
            vec.tensor_mul(out=AT, in0=ExpT, in1=Sinv)
            if parts == "soft":
                return

            # mix: token t=4c+j -> mixT[e, h] = XV_t^T @ AT_t
            # bank j holds its 16 tokens at col-slot c (concurrent row-tiles
            # must drain into DIFFERENT psum banks)
            mbs = [mx_ps.tile([128, 512], F32, tag="mx", name=f"mb_{g}_{j}")
                   for j in range(4)]
            for c in range(16):
                for j in range(4):
                    nc.tensor.matmul(
                        mbs[j][:, 32 * c:32 * (c + 1)],
                        XV[32 * j:32 * (j + 1), 128 * c:128 * (c + 1)],
                        AT[32 * j:32 * (j + 1), 32 * c:32 * (c + 1)],
                        start=True, stop=True, tile_position=(32 * j, 0))
            # drain: bank j [e, (c16, h)] -> attnT half [e, h, (c,j)]
            half, gh = attnT_halves[(64 * g) // 512], (64 * g) % 512 // 64
            half_q = half.rearrange("p h (a r) -> p h a r", r=4)
            for j in range(4):
                copy_op(vec if j % 2 == 0 else sca,
                        half_q[:, :, 16 * gh:16 * (gh + 1), j],
                        mbs[j].rearrange("p (c h) -> p h c", c=16))

        # ---- phase 2: yT = sum_h W2(h)^T @ attnT + bo, W2 read ONCE ----
        # wt arrives as ONE flat DMA (2KB+ contiguous per partition; <512B
        # descriptors pay 2x) issued from the idle Pool queue to keep SP free.
        # Both token-halves accumulate simultaneously in a 2-bank pair.
        def emit_p2_chunk(fc, attnT_halves):
            pb = [mm_ps.tile([128, 512], F32, tag="mm", name=f"p2_{fc}_{t}")
                  for t in range(TH)]
            wt = p2_sb.tile([128, H * 128], BF16, tag="wt")
            nc.gpsimd.dma_start(wt, w2b[fc, :, :])
            for h in range(H):
                for t in range(TH):
                    nc.tensor.matmul(pb[t], wt[:, 128 * h:128 * (h + 1)],
                                     attnT_halves[t][:, h, :],
                                     start=(h == 0), stop=(h == H - 1))
            for t in range(TH):
                yt_sb = out_sb.tile([128, 512], BF16, tag="yt")
                sca.activation(yt_sb, pb[t], AF.Identity,
                               bias=bo_sb[:, fc:fc + 1], scale=1.0)
                nc.sync.dma_start(
                    yT[128 * fc:128 * (fc + 1), 512 * t:512 * (t + 1)], yt_sb)

        # ---- emission: interleave phase-2 chunks once their tokens exist ----
        for rep in range(repeat):
            attnT_halves = [attnT_pool.tile([128, H, 512], BF16,
                                            name=f"attnT_{i}_{rep}", tag=f"at{i}")
                            for i in range(TH)]
            ci = 0
            if parts == "p2only":
                for hl in attnT_halves:
                    nc.vector.memset(hl, 0.5)
                fronts = [None] * G
            else:
                fronts = [None] * G
                fronts[0] = emit_front(0)
            for g in range(G):
                if parts != "p2only" and g + 1 < G:
                    fronts[g + 1] = emit_front(g + 1)
                emit_back(fronts[g], attnT_halves)
                fronts[g] = None
            if parts in ("all", "p2only", "seq"):
                for fc in range(FCH_):
                    emit_p2_chunk(fc, attnT_halves)

    nc.compile()
    return nc


_nc_cache = {}


def _get_nc(TPC, repeat=1):
    key = (TPC, repeat)
    if key not in _nc_cache:
        _nc_cache[key] = _build(TPC, repeat)
    return _nc_cache[key]


def make_in_maps(query, key, value, mask, Wq, Wk, Wv, Wo, bo):
    """Shard + host-prep the full inputs into per-core input maps."""
    B, S, dm = query.shape
    T = B * S
    TPC = T // N_CORES
    xq = np.asarray(query, np.float32).reshape(T, dm)
    xk = np.asarray(key, np.float32).reshape(T, dm)
    xv = np.asarray(value, np.float32).reshape(T, dm)
    m32 = np.asarray(mask, np.float32).reshape(H, H)
    Wqf = np.asarray(Wq, np.float32)
    Wkf = np.asarray(Wk, np.float32)
    Wvf = np.asarray(Wv, np.float32)
    Wof = np.asarray(Wo, np.float32)

    # C = Wq^T Wk / sqrt(d_model); device lhsT = C^T
    ct = np.ascontiguousarray((Wkf.T @ Wqf / np.sqrt(dm)).astype(BF))
    # blockdiag ones [k, p]: 1 where same 32-block
    blkd = np.kron(np.eye(4, dtype=np.float32),
                   np.ones((32, 32), np.float32)).astype(BF)
    # E^T bank layout: [(j,t') parts, (c,h) cols] -> mask^T tiled
    maskt = np.ascontiguousarray(np.tile(m32.T, (4, 16)).astype(np.float32))
    # W2 = blockdiag(Wv^T) @ Wo^T : [(h,e), f]
    W2 = (Wof.T.reshape(H, D, dm).transpose(0, 2, 1) @ Wvf).transpose(0, 2, 1)
    W2 = W2.reshape(dm, dm)  # [(h,e), f]
    w2b = np.ascontiguousarray(
        W2.astype(BF).reshape(H, 128, dm // 128, 128)
        .transpose(2, 1, 0, 3).reshape(dm // 128, 128, H * 128))
    bo32 = np.ascontiguousarray(
        np.asarray(bo, np.float32).reshape(FCH_, 128).T)
    shared = {"ct": ct, "blkd": blkd, "maskt": maskt, "w2b": w2b, "bo32": bo32}

    in_maps = []
    for c in range(N_CORES):
        sl = slice(c * TPC, (c + 1) * TPC)
        xvc = xv[sl].reshape(TPC // 4, 4, H, D).transpose(1, 2, 0, 3)
        in_maps.append({
            "xqb": np.ascontiguousarray(xq[sl].reshape(TPC * H, D).astype(BF).T),
            "xkb": np.ascontiguousarray(xk[sl].reshape(TPC * H, D).astype(BF).T),
            "xvb": np.ascontiguousarray(xvc.reshape(D, TPC * H).astype(BF)),
            **shared,
        })
    return in_maps, TPC


def kernel(query, key, value, mask, Wq, Wk, Wv, Wo, bo):
    from concourse.bass_utils import run_bass_kernel_spmd

    B, S, dm = query.shape
    in_maps, TPC = make_in_maps(query, key, value, mask, Wq, Wk, Wv, Wo, bo)
    nc = _get_nc(TPC)
    res = run_bass_kernel_spmd(nc, in_maps, list(range(N_CORES)))
    out = np.empty((B * S, dm), np.float32)
    for c in range(N_CORES):
        out[c * TPC:(c + 1) * TPC] = res.results[c]["yT"].T.astype(np.float32)
    return out.reshape(B, S, dm)
